# revision 3
# baseline (speedup 1.0000x reference)
"""Multi-head attention (B=2, S=4096, D=512, H=8, HD=64, fp32) on 8 TRN2 cores.

Same math and sharding (core c -> batch c//4,
head pair c%4), restructured schedule:

  * consts/weights DMA'd first (packed: one wpack [512,384] bf16 tensor and
    one cpack [128,258] f32 tensor) so the first projection matmul is not
    queued behind the 4 MB xT transfer.
  * PSUM re-budget: scores double-buffer [128,1024]x2 (4 banks, tag sc),
    attention*V accumulators [65,512]x2 (2 banks, tag av), and a separate
    2-bank transient pool (tag tr) for QK-projection / V-projection /
    output-transpose tiles.  In v1 the projections and transposes shared
    slots with the scores / accumulators, which serialized the whole
    projection phase in front of the first exp (ACT idle ~39us at start) and
    stalled ACT ~3us at every q-group boundary.
  * K and V projections are emitted just-in-time inside q-group 0's k-chunk
    loop, so attention (and ScalarE) starts as soon as block 0 is projected.
  * attention*V uses fp8(e4m3) DoubleRow matmuls: exp output is written as
    fp8 pairs [128, 2(kc), 1024], V is stored fp8 as [128, 32, 80] (col 64 =
    1.0 for the softmax row-sum trick), one matmul per kc-PAIR per head with
    perf_mode=DoubleRow (2 MACs/cell/cycle).
"""

import numpy as np

B, S, D, H = 2, 4096, 512, 8
HD = D // H          # 64
OD = 128             # output dims per core (2 heads)
QW = 512             # query group width

_CACHE = {}


def _build(s=S, rep=1, dr=True, loop=1):
    import concourse.bacc as bacc
    import concourse.mybir as mybir
    import concourse.tile as tile

    f32 = mybir.dt.float32
    bf16 = mybir.dt.bfloat16
    f8 = mybir.dt.float8e4

    nc = bacc.Bacc(None, target_bir_lowering=False)

    xT = nc.dram_tensor("xT", [D, s], bf16, kind="ExternalInput")
    wpack = nc.dram_tensor("wpack", [D, 384], bf16, kind="ExternalInput")
    cpack = nc.dram_tensor("cpack", [128, 258], f32, kind="ExternalInput")
    out = nc.dram_tensor("out", [s, OD], f32, kind="ExternalOutput")

    with tile.TileContext(nc) as tc:
        with (
            tc.tile_pool(name="persist", bufs=1) as persist,
            tc.tile_pool(name="exps", bufs=3) as exps,
            tc.tile_pool(name="outsb", bufs=8) as outsb,
            tc.tile_pool(name="outt", bufs=4) as outtp,
            tc.tile_pool(name="pssc", bufs=2, space="PSUM") as pssc,
            tc.tile_pool(name="psav", bufs=2, space="PSUM") as psav,
            tc.tile_pool(name="pstr", bufs=2, space="PSUM") as pstr,
        ):
            Exp = mybir.ActivationFunctionType.Exp
            # Warm the exp table during input DMA.
            warm = persist.tile([1, 1], f32, name="warm", tag="warm")
            nc.vector.memset(warm[:], 0.0)
            nc.scalar.activation(warm[:], warm[:], Exp)
            # Warm the PE (HAM clock ramps to 2.4 GHz only after ~3us of
            # continuous busy): stream dummy matmuls on a zeroed tile while
            # the input DMAs are in flight, so the first projections run at
            # full clock.  The tiny copy afterwards keeps the chain from
            # being DCE'd.
            dummy = persist.tile([128, 64], bf16, name="dummy", tag="dummy")
            nc.vector.memset(dummy[:], 0.0)
            dps = pstr.tile([128, 64], f32, name="ps_tr", tag="tr")
            for i in range(40):
                nc.tensor.matmul(dps[0:64, :], lhsT=dummy[:], rhs=dummy[:],
                                 start=(i == 0), stop=(i == 39))
            nc.vector.tensor_copy(dummy[:, 0:1], dps[:, 0:1])

            # ---- consts first (small), then x ----
            wp = [persist.tile([128, 384], bf16, name=f"wp{c}", tag=f"wp{c}")
                  for c in range(4)]
            xt = [persist.tile([128, s], bf16, name=f"xt{c}", tag=f"xt{c}")
                  for c in range(4)]
            # Interleave the small weight DMAs with the first x chunk so the
            # first projection matmul's operands all land ASAP.
            for c in range(4):
                nc.sync.dma_start(wp[c][:], wpack[c * 128:(c + 1) * 128, :])
                nc.sync.dma_start(xt[c][:, 0:QW],
                                  xT[c * 128:(c + 1) * 128, 0:QW])
            cp = persist.tile([128, 258], f32, name="cp", tag="cp")
            nc.sync.dma_start(cp[:], cpack[:])
            bk_t = cp[:, 0:1]
            bq_t = cp[:, 1:2]
            bvb_t = cp[:, 2:130]
            id_t = cp[:, 130:258]

            h2 = (s - QW) // 2 + QW
            for lo, hi in ((QW, h2), (h2, s)):
                for c in range(4):
                    nc.sync.dma_start(xt[c][:, lo:hi],
                                      xT[c * 128:(c + 1) * 128, lo:hi])

            qt = persist.tile([128, s], bf16, name="qt", tag="qt")
            kt = persist.tile([128, s], bf16, name="kt", tag="kt")
            if dr:
                v_sb = [persist.tile([128, 32, 80], f8, name=f"vsb{h}",
                                     tag=f"vsb{h}") for h in (0, 1)]
            else:
                v_sb = [persist.tile([128, 32, 65], bf16, name=f"vsb{h}",
                                     tag=f"vsb{h}") for h in (0, 1)]
            for h in (0, 1):
                nc.vector.memset(v_sb[h][:], 1.0)

            def body():
                _emit_body(nc, tc, mybir, s, dr, qt, kt, xt, wp,
                           bq_t, bk_t, bvb_t, id_t, v_sb, out,
                           exps, outsb, outtp, pssc, psav, pstr)

            if loop > 1:
                # Hardware loop around the body: used only for benchmarking
                # (re-runs the identical computation; amplifies kernel time
                # over the per-call host/transfer overhead).
                with tc.For_i(0, loop, 1):
                    body()
            else:
                for _ in range(rep):
                    body()

    nc.compile()
    return nc


def _emit_body(nc, tc, mybir, s, dr, qt, kt, xt, wp, bq_t, bk_t, bvb_t, id_t,
               v_sb, out, exps, outsb, outtp, pssc, psav, pstr):
    f32 = mybir.dt.float32
    bf16 = mybir.dt.bfloat16
    f8 = mybir.dt.float8e4
    Exp = mybir.ActivationFunctionType.Exp
    DR = mybir.MatmulPerfMode.DoubleRow
    qg_n = s // QW
    kc_n = s // 128

    def proj_qk(dst, wcol, b_t, sb):
        ps = pstr.tile([128, QW], f32, name="ps_tr", tag="tr")
        for c in range(4):
            nc.tensor.matmul(
                ps[:],
                lhsT=wp[c][:, wcol:wcol + 128],
                rhs=xt[c][:, sb * QW:(sb + 1) * QW],
                start=(c == 0),
                stop=(c == 3),
            )
        nc.vector.tensor_scalar_add(
            dst[:, sb * QW:(sb + 1) * QW], ps[:], b_t
        )

    def vchunk(j):
        ps = pstr.tile([128, 128], f32, name="ps_tr", tag="tr")
        for c in range(4):
            nc.tensor.matmul(
                ps[:],
                lhsT=xt[c][:, j * 128:(j + 1) * 128],
                rhs=wp[c][:, 256:384],
                start=(c == 0),
                stop=(c == 3),
            )
        for h in (0, 1):
            nc.vector.tensor_add(
                v_sb[h][:, j:j + 1, 0:64],
                ps[:, h * 64:(h + 1) * 64],
                bvb_t[:, h * 64:(h + 1) * 64],
            )

    # K block 0 and Q block 0 up front; V and remaining K just-in-time in
    # qg 0 (emitted after each chunk's scores+exp so the score pipeline that
    # feeds ScalarE always has priority on the PE).
    proj_qk(kt, 0, bk_t, 0)
    proj_qk(qt, 128, bq_t, 0)

    groups = [(g * QW, QW, g == 0, g + 1 if g + 1 < qg_n else None)
              for g in range(qg_n)]

    for q0, qw, first, qproj in groups:
        av = [psav.tile([65, qw], f32, name="av", tag="av") for _ in (0, 1)]
        expair = None
        for kc in range(kc_n):
            # scores for both heads into one [128, 2*qw] PSUM tile
            ps = pssc.tile([128, 2, qw], f32, name="ps_sc", tag="sc")
            for h in (0, 1):
                nc.tensor.matmul(
                    ps[:, h:h + 1, :],
                    lhsT=kt[h * HD:(h + 1) * HD, kc * 128:(kc + 1) * 128],
                    rhs=qt[h * HD:(h + 1) * HD, q0:q0 + qw],
                    start=True,
                    stop=True,
                )
            if first:
                if kc % 4 == 2 and kc < 28:
                    proj_qk(kt, 0, bk_t, kc // 4 + 1)
                if kc < 2:
                    vchunk(kc)
                if kc < kc_n - 2:
                    vchunk(kc + 2)
            if kc == 8 and qproj is not None:
                proj_qk(qt, 128, bq_t, qproj)
            if dr:
                if kc % 2 == 0:
                    expair = exps.tile([128, 2, 2 * qw], f8, name="ex",
                                       tag="exp")
                nc.scalar.activation(expair[:, kc % 2:kc % 2 + 1, :], ps[:],
                                     Exp)
                if kc % 2 == 1:
                    for h in (0, 1):
                        nc.tensor.matmul(
                            av[h][:],
                            lhsT=v_sb[h][:, kc - 1:kc + 1, 0:65],
                            rhs=expair[:, :, h * qw:(h + 1) * qw],
                            start=(kc == 1),
                            stop=(kc == kc_n - 1),
                            perf_mode=DR,
                        )
            else:
                ex = exps.tile([128, 2, qw], bf16, name="ex", tag="exp")
                nc.scalar.activation(ex[:], ps[:], Exp)
                for h in (0, 1):
                    nc.tensor.matmul(
                        av[h][:],
                        lhsT=v_sb[h][:, kc, 0:65],
                        rhs=ex[:, h:h + 1, :],
                        start=(kc == 0),
                        stop=(kc == kc_n - 1),
                    )
        # tail: both copies first (releasing the av slots), then per-block
        # transpose + normalize + store.  Transposes recycle the freed av
        # slots so the transient pool stays clear for JIT projections.
        nblk = qw // 128
        ot = [outsb.tile([128, OD], f32, name="ot", tag="outsb")
              for _ in range(nblk)]
        outt = [outtp.tile([65, qw], f32, name="outt", tag="outt")
                for _ in (0, 1)]
        for h in (0, 1):
            nc.vector.tensor_copy(outt[h][:], av[h][:])
        for blk in range(nblk):
            for h in (0, 1):
                tp = psav.tile([128, 65], f32, name="tp", tag="av")
                nc.tensor.transpose(
                    tp[:],
                    outt[h][:, blk * 128:(blk + 1) * 128],
                    id_t[0:65, 0:65],
                )
                rs = outsb.tile([128, 1], f32, name="rs", tag="rs")
                nc.vector.reciprocal(rs[:], tp[:, 64:65])
                nc.vector.tensor_scalar_mul(
                    ot[blk][:, h * HD:(h + 1) * HD], tp[:, 0:64], rs[:]
                )
            r0 = q0 + blk * 128
            nc.sync.dma_start(out[r0:r0 + 128, :], ot[blk][:])


def _get_nc(s=S):
    if s not in _CACHE:
        _CACHE[s] = _build(s)
    return _CACHE[s]


def _shard_inputs(x, Wq, bq, Wk, bk, Wv, bv):
    import ml_dtypes

    bf16 = ml_dtypes.bfloat16
    f32 = np.float32
    ident = np.eye(128, dtype=f32)
    xTb = [np.ascontiguousarray(x[b].T).astype(bf16) for b in range(B)]
    wpacks, cpacks = [], []
    for hp in range(4):
        r = slice(128 * hp, 128 * hp + 128)
        wpk = np.concatenate(
            [Wk[r].T, (Wq[r] * 0.125).T, Wv[r].T], axis=1)  # [512, 384]
        wpacks.append(np.ascontiguousarray(wpk).astype(bf16))
        cpk = np.concatenate(
            [bk[r].reshape(128, 1), (bq[r] * 0.125).reshape(128, 1),
             np.tile(bv[r][None, :], (128, 1)), ident], axis=1)  # [128, 258]
        cpacks.append(np.ascontiguousarray(cpk).astype(f32))
    in_maps = []
    for c in range(8):
        b, hp = divmod(c, 4)
        in_maps.append({
            "xT": xTb[b],
            "wpack": wpacks[hp],
            "cpack": cpacks[hp],
        })
    return in_maps


def kernel(x, Wq, bq, Wk, bk, Wv, bv, _trace=False):
    from concourse.bass_utils import run_bass_kernel_spmd

    x = np.asarray(x, dtype=np.float32)
    Wq = np.asarray(Wq, dtype=np.float32)
    bq = np.asarray(bq, dtype=np.float32)
    Wk = np.asarray(Wk, dtype=np.float32)
    bk = np.asarray(bk, dtype=np.float32)
    Wv = np.asarray(Wv, dtype=np.float32)
    bv = np.asarray(bv, dtype=np.float32)

    nc = _get_nc(S)
    in_maps = _shard_inputs(x, Wq, bq, Wk, bk, Wv, bv)
    res = run_bass_kernel_spmd(nc, in_maps, core_ids=list(range(8)),
                               trace=_trace)
    kernel._last_results = res

    out = np.empty((B, S, D), dtype=np.float32)
    for c in range(8):
        b, hp = divmod(c, 4)
        out[b, :, 128 * hp:128 * hp + 128] = res.results[c]["out"]
    return out


# revision 4
# speedup vs baseline: 1.2520x; 1.2520x over previous
"""Multi-head attention (B=2, S=4096, D=512, H=8, HD=64, fp32) on 8 TRN2 cores.

Same math and sharding (core c -> batch c//4,
head pair c%4), restructured schedule:

  * consts/weights DMA'd first (packed: one wpack [512,384] bf16 tensor and
    one cpack [128,258] f32 tensor) so the first projection matmul is not
    queued behind the 4 MB xT transfer.
  * PSUM re-budget: scores double-buffer [128,1024]x2 (4 banks, tag sc),
    attention*V accumulators [65,512]x2 (2 banks, tag av), and a separate
    2-bank transient pool (tag tr) for QK-projection / V-projection /
    output-transpose tiles.  In v1 the projections and transposes shared
    slots with the scores / accumulators, which serialized the whole
    projection phase in front of the first exp (ACT idle ~39us at start) and
    stalled ACT ~3us at every q-group boundary.
  * K and V projections are emitted just-in-time inside q-group 0's k-chunk
    loop, so attention (and ScalarE) starts as soon as block 0 is projected.
  * attention*V uses fp8(e4m3) DoubleRow matmuls: exp output is written as
    fp8 pairs [128, 2(kc), 1024], V is stored fp8 as [128, 32, 80] (col 64 =
    1.0 for the softmax row-sum trick), one matmul per kc-PAIR per head with
    perf_mode=DoubleRow (2 MACs/cell/cycle).
"""

import numpy as np

B, S, D, H = 2, 4096, 512, 8
HD = D // H          # 64
OD = 128             # output dims per core (2 heads)
QW = 512             # query group width

_CACHE = {}


def _build(s=S, rep=1, dr=True, loop=1, empty=False):
    import concourse.bacc as bacc
    import concourse.mybir as mybir
    import concourse.tile as tile

    f32 = mybir.dt.float32
    bf16 = mybir.dt.bfloat16
    f8 = mybir.dt.float8e4

    nc = bacc.Bacc(None, target_bir_lowering=False)

    xT = nc.dram_tensor("xT", [D, s], bf16, kind="ExternalInput")
    wpack = nc.dram_tensor("wpack", [D, 384], bf16, kind="ExternalInput")
    cpack = nc.dram_tensor("cpack", [128, 258], f32, kind="ExternalInput")
    out = nc.dram_tensor("out", [s, OD], f32, kind="ExternalOutput")

    with tile.TileContext(nc) as tc:
        with (
            tc.tile_pool(name="persist", bufs=1) as persist,
            tc.tile_pool(name="exps", bufs=3) as exps,
            tc.tile_pool(name="outsb", bufs=8) as outsb,
            tc.tile_pool(name="outt", bufs=4) as outtp,
            tc.tile_pool(name="pssc", bufs=2, space="PSUM") as pssc,
            tc.tile_pool(name="psav", bufs=2, space="PSUM") as psav,
            tc.tile_pool(name="pstr", bufs=2, space="PSUM") as pstr,
        ):
            Exp = mybir.ActivationFunctionType.Exp
            # Warm the exp table during input DMA.
            warm = persist.tile([1, 1], f32, name="warm", tag="warm")
            nc.vector.memset(warm[:], 0.0)
            nc.scalar.activation(warm[:], warm[:], Exp)
            # Warm the PE (HAM clock ramps to 2.4 GHz only after ~3us of
            # continuous busy): stream dummy matmuls on a zeroed tile while
            # the input DMAs are in flight, so the first projections run at
            # full clock.  The tiny copy afterwards keeps the chain from
            # being DCE'd.
            dummy = persist.tile([128, 64], bf16, name="dummy", tag="dummy")
            nc.vector.memset(dummy[:], 0.0)
            dps = pstr.tile([128, 64], f32, name="ps_tr", tag="tr")
            for i in range(40):
                nc.tensor.matmul(dps[0:64, :], lhsT=dummy[:], rhs=dummy[:],
                                 start=(i == 0), stop=(i == 39))
            nc.vector.tensor_copy(dummy[:, 0:1], dps[:, 0:1])

            # ---- consts first (small), then x ----
            wp = [persist.tile([128, 384], bf16, name=f"wp{c}", tag=f"wp{c}")
                  for c in range(4)]
            xt = [persist.tile([128, s], bf16, name=f"xt{c}", tag=f"xt{c}")
                  for c in range(4)]
            # Interleave the small weight DMAs with the first x chunk so the
            # first projection matmul's operands all land ASAP.
            for c in range(4):
                nc.sync.dma_start(wp[c][:], wpack[c * 128:(c + 1) * 128, :])
                nc.sync.dma_start(xt[c][:, 0:QW],
                                  xT[c * 128:(c + 1) * 128, 0:QW])
            cp = persist.tile([128, 258], f32, name="cp", tag="cp")
            nc.sync.dma_start(cp[:], cpack[:])
            bk_t = cp[:, 0:1]
            bq_t = cp[:, 1:2]
            bvb_t = cp[:, 2:130]
            id_t = cp[:, 130:258]

            h2 = (s - QW) // 2 + QW
            for lo, hi in ((QW, h2), (h2, s)):
                for c in range(4):
                    nc.sync.dma_start(xt[c][:, lo:hi],
                                      xT[c * 128:(c + 1) * 128, lo:hi])

            qt = persist.tile([128, s], bf16, name="qt", tag="qt")
            kt = persist.tile([128, s], bf16, name="kt", tag="kt")
            if dr:
                v_sb = [persist.tile([128, 32, 80], f8, name=f"vsb{h}",
                                     tag=f"vsb{h}") for h in (0, 1)]
            else:
                v_sb = [persist.tile([128, 32, 65], bf16, name=f"vsb{h}",
                                     tag=f"vsb{h}") for h in (0, 1)]
            for h in (0, 1):
                nc.vector.memset(v_sb[h][:], 1.0)

            def body():
                _emit_body(nc, tc, mybir, s, dr, qt, kt, xt, wp,
                           bq_t, bk_t, bvb_t, id_t, v_sb, out,
                           exps, outsb, outtp, pssc, psav, pstr)

            if loop > 1:
                # Hardware loop around the body: used only for benchmarking
                # (re-runs the identical computation; amplifies kernel time
                # over the per-call host/transfer overhead).  empty=True
                # keeps only a token op in the loop, to measure the For_i
                # per-iteration barrier overhead for subtraction.
                with tc.For_i(0, loop, 1):
                    if empty:
                        nc.vector.memset(warm[:], 0.0)
                    else:
                        body()
            else:
                for _ in range(rep):
                    body()

    nc.compile()
    return nc


def _emit_body(nc, tc, mybir, s, dr, qt, kt, xt, wp, bq_t, bk_t, bvb_t, id_t,
               v_sb, out, exps, outsb, outtp, pssc, psav, pstr):
    f32 = mybir.dt.float32
    bf16 = mybir.dt.bfloat16
    f8 = mybir.dt.float8e4
    Exp = mybir.ActivationFunctionType.Exp
    DR = mybir.MatmulPerfMode.DoubleRow
    qg_n = s // QW
    kc_n = s // 128

    def proj_qk(dst, wcol, b_t, sb):
        ps = pstr.tile([128, QW], f32, name="ps_tr", tag="tr")
        for c in range(4):
            nc.tensor.matmul(
                ps[:],
                lhsT=wp[c][:, wcol:wcol + 128],
                rhs=xt[c][:, sb * QW:(sb + 1) * QW],
                start=(c == 0),
                stop=(c == 3),
            )
        nc.vector.tensor_scalar_add(
            dst[:, sb * QW:(sb + 1) * QW], ps[:], b_t
        )

    def vchunk(j):
        ps = pstr.tile([128, 128], f32, name="ps_tr", tag="tr")
        for c in range(4):
            nc.tensor.matmul(
                ps[:],
                lhsT=xt[c][:, j * 128:(j + 1) * 128],
                rhs=wp[c][:, 256:384],
                start=(c == 0),
                stop=(c == 3),
            )
        for h in (0, 1):
            nc.vector.tensor_add(
                v_sb[h][:, j:j + 1, 0:64],
                ps[:, h * 64:(h + 1) * 64],
                bvb_t[:, h * 64:(h + 1) * 64],
            )

    # K block 0 and Q block 0 up front; V and remaining K just-in-time in
    # qg 0 (emitted after each chunk's scores+exp so the score pipeline that
    # feeds ScalarE always has priority on the PE).
    proj_qk(kt, 0, bk_t, 0)
    proj_qk(qt, 128, bq_t, 0)

    groups = [(g * QW, QW, g == 0, g + 1 if g + 1 < qg_n else None)
              for g in range(qg_n)]

    for q0, qw, first, qproj in groups:
        av = [psav.tile([65, qw], f32, name="av", tag="av") for _ in (0, 1)]
        expair = None
        for kc in range(kc_n):
            # scores for both heads into one [128, 2*qw] PSUM tile
            ps = pssc.tile([128, 2, qw], f32, name="ps_sc", tag="sc")
            for h in (0, 1):
                nc.tensor.matmul(
                    ps[:, h:h + 1, :],
                    lhsT=kt[h * HD:(h + 1) * HD, kc * 128:(kc + 1) * 128],
                    rhs=qt[h * HD:(h + 1) * HD, q0:q0 + qw],
                    start=True,
                    stop=True,
                )
            if first:
                if kc % 4 == 2 and kc < 28:
                    proj_qk(kt, 0, bk_t, kc // 4 + 1)
                if kc < 2:
                    vchunk(kc)
                if kc < kc_n - 2:
                    vchunk(kc + 2)
            if kc == 8 and qproj is not None:
                proj_qk(qt, 128, bq_t, qproj)
            if dr:
                if kc % 2 == 0:
                    expair = exps.tile([128, 2, 2 * qw], f8, name="ex",
                                       tag="exp")
                nc.scalar.activation(expair[:, kc % 2:kc % 2 + 1, :], ps[:],
                                     Exp)
                if kc % 2 == 1:
                    for h in (0, 1):
                        nc.tensor.matmul(
                            av[h][:],
                            lhsT=v_sb[h][:, kc - 1:kc + 1, 0:65],
                            rhs=expair[:, :, h * qw:(h + 1) * qw],
                            start=(kc == 1),
                            stop=(kc == kc_n - 1),
                            perf_mode=DR,
                        )
            else:
                ex = exps.tile([128, 2, qw], bf16, name="ex", tag="exp")
                nc.scalar.activation(ex[:], ps[:], Exp)
                for h in (0, 1):
                    nc.tensor.matmul(
                        av[h][:],
                        lhsT=v_sb[h][:, kc, 0:65],
                        rhs=ex[:, h:h + 1, :],
                        start=(kc == 0),
                        stop=(kc == kc_n - 1),
                    )
        # tail: both copies first (releasing the av slots), then per-block
        # transpose + normalize + store.  Transposes recycle the freed av
        # slots so the transient pool stays clear for JIT projections.
        nblk = qw // 128
        ot = [outsb.tile([128, OD], f32, name="ot", tag="outsb")
              for _ in range(nblk)]
        outt = [outtp.tile([65, qw], f32, name="outt", tag="outt")
                for _ in (0, 1)]
        for h in (0, 1):
            nc.vector.tensor_copy(outt[h][:], av[h][:])
        for blk in range(nblk):
            for h in (0, 1):
                tp = psav.tile([128, 65], f32, name="tp", tag="av")
                nc.tensor.transpose(
                    tp[:],
                    outt[h][:, blk * 128:(blk + 1) * 128],
                    id_t[0:65, 0:65],
                )
                rs = outsb.tile([128, 1], f32, name="rs", tag="rs")
                nc.vector.reciprocal(rs[:], tp[:, 64:65])
                nc.vector.tensor_scalar_mul(
                    ot[blk][:, h * HD:(h + 1) * HD], tp[:, 0:64], rs[:]
                )
            r0 = q0 + blk * 128
            nc.sync.dma_start(out[r0:r0 + 128, :], ot[blk][:])


def _get_nc(s=S):
    if s not in _CACHE:
        _CACHE[s] = _build(s)
    return _CACHE[s]


def _shard_inputs(x, Wq, bq, Wk, bk, Wv, bv):
    import ml_dtypes

    bf16 = ml_dtypes.bfloat16
    f32 = np.float32
    ident = np.eye(128, dtype=f32)
    xTb = [np.ascontiguousarray(x[b].T).astype(bf16) for b in range(B)]
    wpacks, cpacks = [], []
    for hp in range(4):
        r = slice(128 * hp, 128 * hp + 128)
        wpk = np.concatenate(
            [Wk[r].T, (Wq[r] * 0.125).T, Wv[r].T], axis=1)  # [512, 384]
        wpacks.append(np.ascontiguousarray(wpk).astype(bf16))
        cpk = np.concatenate(
            [bk[r].reshape(128, 1), (bq[r] * 0.125).reshape(128, 1),
             np.tile(bv[r][None, :], (128, 1)), ident], axis=1)  # [128, 258]
        cpacks.append(np.ascontiguousarray(cpk).astype(f32))
    in_maps = []
    for c in range(8):
        b, hp = divmod(c, 4)
        in_maps.append({
            "xT": xTb[b],
            "wpack": wpacks[hp],
            "cpack": cpacks[hp],
        })
    return in_maps


def kernel(x, Wq, bq, Wk, bk, Wv, bv, _trace=False):
    from concourse.bass_utils import run_bass_kernel_spmd

    x = np.asarray(x, dtype=np.float32)
    Wq = np.asarray(Wq, dtype=np.float32)
    bq = np.asarray(bq, dtype=np.float32)
    Wk = np.asarray(Wk, dtype=np.float32)
    bk = np.asarray(bk, dtype=np.float32)
    Wv = np.asarray(Wv, dtype=np.float32)
    bv = np.asarray(bv, dtype=np.float32)

    nc = _get_nc(S)
    in_maps = _shard_inputs(x, Wq, bq, Wk, bk, Wv, bv)
    res = run_bass_kernel_spmd(nc, in_maps, core_ids=list(range(8)),
                               trace=_trace)
    kernel._last_results = res

    out = np.empty((B, S, D), dtype=np.float32)
    for c in range(8):
        b, hp = divmod(c, 4)
        out[b, :, 128 * hp:128 * hp + 128] = res.results[c]["out"]
    return out


# revision 11
# speedup vs baseline: 1.3079x; 1.0447x over previous
"""Multi-head attention (B=2, S=4096, D=512, H=8, HD=64, fp32) on 8 TRN2 cores.

Same math and sharding (core c -> batch c//4,
head pair c%4), restructured schedule:

  * consts/weights/x-block-0 DMA'd first in single dispatches (wpack
    [128,1536] bf16 packed K|Q|V-major so K's weights land first, xA
    [128,2048] bf16 = the four partition-chunks of x cols 0:512 side by
    side, cpack [128,258] f32) so the first projection is not queued behind
    the 4 MB xT transfer; V-tile memsets go to the idle GpSimd.
  * two-phase PSUM budget (Tile pools are lifetime-scoped, so a closed
    pool's banks are recycled): phase 1 = q-groups 0-1 with narrow
    [128,2,512]x2 score tiles (4 banks) + a 2-slot transient pool for the
    PE-clock warmup and the just-in-time K/V/Q projections (K and V stream
    inside qg0's k-chunk loop; Q blocks 2-7 inside qg1's, where the PE has
    slack); phase 2 = q-groups 2-7 with [128,3,512]x2 score tiles (6
    banks), so each exp instruction covers 1536 elements - fewer
    instructions on the bottleneck ScalarE.  A token-dependency barrier
    gates phase 2's first PE write behind phase 1's final exp (the bank
    recycling is safe only once the last reader retires).
  * exp output streams into a persistent full-q-group fp8 ring
    [128, 32768]; attention*V uses fp8(e4m3) DoubleRow matmuls reading
    kc-pair regions as [128, 2, 512] views (AP.rearrange), V stored fp8 as
    [128, 32, 80] with col 64 = 1.0 (the softmax row-sums ride along in the
    matmul), one matmul per kc-pair per head (2 MACs/cell/cycle).
  * the last q-group is head-staggered (head 1's 32 k-chunks sweep first),
    so head 1's transpose/normalize tail hides under head 0's exps and
    only half a tail remains exposed after the final exp.
"""

import numpy as np

B, S, D, H = 2, 4096, 512, 8
HD = D // H          # 64
OD = 128             # output dims per core (2 heads)
QW = 512             # query group width

_CACHE = {}


def _build(s=S, rep=1, dr=True, loop=1, empty=False, fdu2=3):
    import concourse.bacc as bacc
    import concourse.mybir as mybir
    import concourse.tile as tile

    f32 = mybir.dt.float32
    bf16 = mybir.dt.bfloat16
    f8 = mybir.dt.float8e4

    nc = bacc.Bacc(None, target_bir_lowering=False)

    xT = nc.dram_tensor("xT", [D, s], bf16, kind="ExternalInput")
    # xA: the four 128-partition chunks of x's first 512 columns packed
    # side by side, so block 0 lands in ONE DMA dispatch; wpack: all four
    # weight chunks packed likewise.
    xA = nc.dram_tensor("xA", [128, 4 * QW], bf16, kind="ExternalInput")
    wpack = nc.dram_tensor("wpack", [128, 4 * 384], bf16,
                           kind="ExternalInput")
    cpack = nc.dram_tensor("cpack", [128, 258], f32, kind="ExternalInput")
    out = nc.dram_tensor("out", [s, OD], f32, kind="ExternalOutput")

    with tile.TileContext(nc) as tc:
        with (
            tc.tile_pool(name="persist", bufs=1) as persist,
            tc.tile_pool(name="outsb", bufs=8) as outsb,
            tc.tile_pool(name="outt", bufs=4) as outtp,
            tc.tile_pool(name="psav", bufs=2, space="PSUM") as psav,
        ):
            Exp = mybir.ActivationFunctionType.Exp
            dummy = persist.tile([128, 64], bf16, name="dummy", tag="dummy")
            nc.vector.memset(dummy[:], 0.0)
            # Warm the exp table during input DMA.
            warm = persist.tile([1, 1], f32, name="warm", tag="warm")
            nc.vector.memset(warm[:], 0.0)
            nc.scalar.activation(warm[:], warm[:], Exp)
            # Warm the PE (HAM clock ramps to 2.4 GHz only after ~3us of
            # continuous busy): stream dummy matmuls on a zeroed tile while
            # the input DMAs are in flight, so the first projections run at
            # full clock.  Two chains occupy BOTH transient-pool slots, so
            # the first projection's PSUM allocation (and thus everything
            # after it) queues behind the warmup instead of the scheduler
            # scattering the dummies as gap fillers.  The tiny copies keep
            # the chains from being DCE'd.
            # ---- consts + x block 0 first (one dispatch each), then x ----
            wp_all = persist.tile([128, 4 * 384], bf16, name="wp_all",
                                  tag="wp")
            xa_sb = persist.tile([128, 4 * QW], bf16, name="xa", tag="xa")
            # split so K's weights + x block 0 (the first projection's
            # operands) land before Q/V weights
            nc.sync.dma_start(wp_all[:, 0:512], wpack[:, 0:512])
            nc.sync.dma_start(xa_sb[:], xA[:])
            nc.sync.dma_start(wp_all[:, 512:1024], wpack[:, 512:1024])
            nc.sync.dma_start(wp_all[:, 1024:1536], wpack[:, 1024:1536])
            cp = persist.tile([128, 258], f32, name="cp", tag="cp")
            nc.sync.dma_start(cp[:], cpack[:])
            bk_t = cp[:, 0:1]
            bq_t = cp[:, 1:2]
            bvb_t = cp[:, 2:130]
            id_t = cp[:, 130:258]

            xt = [persist.tile([128, s], bf16, name=f"xt{c}", tag=f"xt{c}")
                  for c in range(4)]
            h2 = (s - QW) // 2 + QW
            for lo, hi in ((QW, h2), (h2, s)):
                for c in range(4):
                    nc.sync.dma_start(xt[c][:, lo:hi],
                                      xT[c * 128:(c + 1) * 128, lo:hi])

            qt = persist.tile([128, s], bf16, name="qt", tag="qt")
            kt = persist.tile([128, s], bf16, name="kt", tag="kt")
            # full-q-group exp ring: every exp instruction writes a
            # contiguous span; attention*V reads kc-pair regions as
            # [128, 2, 512] fp8 views for DoubleRow
            ring = persist.tile([128, 64 * QW], f8, name="ring", tag="ring")
            if dr:
                v_sb = [persist.tile([128, 32, 80], f8, name=f"vsb{h}",
                                     tag=f"vsb{h}") for h in (0, 1)]
            else:
                v_sb = [persist.tile([128, 32, 65], bf16, name=f"vsb{h}",
                                     tag=f"vsb{h}") for h in (0, 1)]
            for h in (0, 1):
                nc.any.memset(v_sb[h][:], 1.0)

            def body():
                _emit_body(nc, tc, mybir, s, dr, qt, kt, xt, xa_sb, wp_all,
                           bq_t, bk_t, bvb_t, id_t, v_sb, ring, out,
                           outsb, outtp, psav, dummy, fdu2)

            if loop > 1:
                # Hardware loop around the body: used only for benchmarking
                # (re-runs the identical computation; amplifies kernel time
                # over the per-call host/transfer overhead).  empty=True
                # keeps only a token op in the loop, to measure the For_i
                # per-iteration barrier overhead for subtraction.
                with tc.For_i(0, loop, 1):
                    if empty:
                        nc.vector.memset(warm[:], 0.0)
                    else:
                        body()
            else:
                for _ in range(rep):
                    body()

    nc.compile()
    return nc


def _emit_body(nc, tc, mybir, s, dr, qt, kt, xt, xa_sb, wp_all, bq_t, bk_t,
               bvb_t, id_t, v_sb, ring, out, outsb, outtp, psav, dummy,
               FDU2=3):
    f32 = mybir.dt.float32
    bf16 = mybir.dt.bfloat16
    f8 = mybir.dt.float8e4
    Exp = mybir.ActivationFunctionType.Exp
    DR = mybir.MatmulPerfMode.DoubleRow
    qg_n = s // QW
    kc_n = s // 128

    def proj_qk(pool, dst, wcol, b_t, sb):
        ps = pool.tile([128, QW], f32, name="ps_tr", tag="tr")
        for c in range(4):
            rhs = (xa_sb[:, c * QW:(c + 1) * QW] if sb == 0 else
                   xt[c][:, sb * QW:(sb + 1) * QW])
            nc.tensor.matmul(
                ps[:],
                lhsT=wp_all[:, wcol * 4 + c * 128:wcol * 4 + (c + 1) * 128],
                rhs=rhs,
                start=(c == 0),
                stop=(c == 3),
            )
        nc.vector.tensor_scalar_add(
            dst[:, sb * QW:(sb + 1) * QW], ps[:], b_t
        )

    def vchunk(pool, j):
        ps = pool.tile([128, 128], f32, name="ps_tr", tag="tr")
        for c in range(4):
            lhsT = (xa_sb[:, c * QW + j * 128:c * QW + (j + 1) * 128]
                    if j < 4 else xt[c][:, j * 128:(j + 1) * 128])
            nc.tensor.matmul(
                ps[:],
                lhsT=lhsT,
                rhs=wp_all[:, 1024 + c * 128:1024 + (c + 1) * 128],
                start=(c == 0),
                stop=(c == 3),
            )
        for h in (0, 1):
            nc.vector.tensor_add(
                v_sb[h][:, j:j + 1, 0:64],
                ps[:, h * 64:(h + 1) * 64],
                bvb_t[:, h * 64:(h + 1) * 64],
            )

    def emit_qg(qg, fdu, pool, trpool, stagger=False):
        """One 512-query group: scores -> exp (fdu units per instruction,
        written into the contiguous exp ring) -> DoubleRow attention*V per
        kc pair.  qg 0 additionally JIT-emits the K/V projections and ALL
        remaining Q-block projections (the later groups' pool has no
        transient slots - their banks belong to the wider score tiles)."""
        q0 = qg * QW
        if stagger:
            # head-major unit order (all of head 1, then head 0): head 1's
            # attention*V completes mid-group, so its output tail hides
            # under head 0's remaining exps and only head 0's tail is
            # exposed after the final exp.  Allocate av[1] first so head
            # 1's transposes can recycle its slot without waiting on the
            # still-accumulating av[0].
            av1 = psav.tile([65, QW], f32, name="av", tag="av")
            av0 = psav.tile([65, QW], f32, name="av", tag="av")
            av = [av0, av1]
            order = [(kc, 1) for kc in range(kc_n)] + \
                    [(kc, 0) for kc in range(kc_n)]
        else:
            av = [psav.tile([65, QW], f32, name="av", tag="av")
                  for _ in (0, 1)]
            order = [(kc, h) for kc in range(kc_n) for h in (0, 1)]
        cur = None
        exp_off = 0
        exp_units = 0
        next_pair = 0
        tiles = {}
        n_units = 2 * kc_n
        last = (qg == qg_n - 1)
        ot = [outsb.tile([128, OD], f32, name="ot", tag="outsb")
              for _ in range(4)]
        tail_done = {0: False, 1: False}

        def tail_head(h):
            # copy (releasing av[h]'s slot), then per-block transpose +
            # normalize.  In the staggered last group head 1's tail runs
            # mid-group on the DVE (ScalarE is still busy with head 0's
            # exps); the final head's copy uses the then-idle ScalarE.
            tail_done[h] = True
            outt = outtp.tile([65, QW], f32, name="outt", tag="outt")
            if last and not stagger or (stagger and h == 0):
                nc.scalar.copy(outt[:], av[h][:])
            elif last and stagger and h == 1:
                nc.vector.tensor_copy(outt[:], av[h][:])
            elif last:
                nc.scalar.copy(outt[:], av[h][:])
            else:
                nc.vector.tensor_copy(outt[:], av[h][:])
            for blk in range(4):
                tp = psav.tile([128, 65], f32, name="tp", tag="av")
                nc.tensor.transpose(
                    tp[:],
                    outt[:, blk * 128:(blk + 1) * 128],
                    id_t[0:65, 0:65],
                )
                rs = outsb.tile([128, 1], f32, name="rs", tag="rs")
                nc.vector.reciprocal(rs[:], tp[:, 64:65])
                nc.vector.tensor_scalar_mul(
                    ot[blk][:, h * HD:(h + 1) * HD], tp[:, 0:64], rs[:]
                )

        def emit_ready(done):
            nonlocal exp_off, exp_units, next_pair
            while (exp_units + fdu <= done or
                   (done == n_units and exp_units < done)):
                n = min(fdu, done - exp_units)
                cur_e = tiles.pop(exp_units // fdu)
                nc.scalar.activation(
                    ring[:, exp_off:exp_off + n * QW],
                    cur_e[:, 0:n, :], Exp)
                exp_off += n * QW
                exp_units += n
                if stagger:
                    while (next_pair + 1) * 2 <= exp_units:
                        P = next_pair
                        hh = 1 if P < kc_n // 2 else 0
                        pih = P % (kc_n // 2)
                        reg = ring[:, P * 1024:(P + 1) * 1024].rearrange(
                            "p (a q) -> p a q", a=2)
                        nc.tensor.matmul(
                            av[hh][:],
                            lhsT=v_sb[hh][:, 2 * pih:2 * pih + 2, 0:65],
                            rhs=reg[:],
                            start=(pih == 0),
                            stop=(pih == kc_n // 2 - 1),
                            perf_mode=DR,
                        )
                        next_pair += 1
                else:
                    while (next_pair + 1) * 4 <= exp_units:
                        p = next_pair
                        reg = ring[:, p * 2048:(p + 1) * 2048].rearrange(
                            "p (a q) -> p a q", a=2)
                        for hh in (0, 1):
                            nc.tensor.matmul(
                                av[hh][:],
                                lhsT=v_sb[hh][:, 2 * p:2 * p + 2, 0:65],
                                rhs=reg[:, :, hh * QW:(hh + 1) * QW],
                                start=(p == 0),
                                stop=(p == kc_n // 2 - 1),
                                perf_mode=DR,
                            )
                        next_pair += 1

        for u, (kc, h) in enumerate(order):
            if u % fdu == 0:
                cur = pool.tile([128, fdu, QW], f32, name="ps_sc",
                                tag="sc")
                tiles[u // fdu] = cur
            nc.tensor.matmul(
                cur[:, (u % fdu):(u % fdu) + 1, :],
                lhsT=kt[h * HD:(h + 1) * HD, kc * 128:(kc + 1) * 128],
                rhs=qt[h * HD:(h + 1) * HD, q0:q0 + QW],
                start=True,
                stop=True,
            )
            # JIT projections sit between the scores and the exp/AV stream:
            # they must precede the AV that consumes them (emission order IS
            # dependency order) without delaying the next kc's scores.
            if trpool is not None and h == 1:
                if qg == 0:
                    if kc % 4 == 2 and kc < 28:
                        proj_qk(trpool, kt, 0, bk_t, kc // 4 + 1)
                    if kc < 2:
                        vchunk(trpool, kc)
                    if kc < kc_n - 2:
                        vchunk(trpool, kc + 2)
                    if kc == 4:
                        proj_qk(trpool, qt, 128, bq_t, 1)
                else:
                    # q-group 1 carries the remaining Q-block projections
                    # (q-group 0's PE is already saturated with K/V JIT)
                    if kc >= 1 and (kc - 1) % 3 == 0 and (kc - 1) // 3 < 6:
                        proj_qk(trpool, qt, 128, bq_t, (kc - 1) // 3 + 2)
            # exps covering completed units, then AV for completed pairs
            emit_ready(u + 1)
            if stagger and next_pair >= kc_n // 2 and not tail_done[1]:
                tail_head(1)
        # end-of-group tail: remaining heads, then the stores
        for h in (0, 1):
            if not tail_done[h]:
                tail_head(h)
        for blk in range(4):
            r0 = q0 + blk * 128
            nc.sync.dma_start(out[r0:r0 + 128, :], ot[blk][:])

    # Phase 1 (q-group 0): narrow score tiles + transient pool for the PE
    # warmup and the JIT K/V/Q projections.
    with (
        tc.tile_pool(name="pssca", bufs=2, space="PSUM") as pssca,
        tc.tile_pool(name="pstr", bufs=2, space="PSUM") as pstr,
    ):
        for ch in range(2):
            dps = pstr.tile([128, 64], f32, name="ps_tr", tag="tr")
            for i in range(28):
                nc.tensor.matmul(dps[0:64, :], lhsT=dummy[:], rhs=dummy[:],
                                 start=(i == 0), stop=(i == 27))
            nc.vector.tensor_copy(dummy[:, ch:ch + 1], dps[:, 0:1])
        proj_qk(pstr, kt, 0, bk_t, 0)
        proj_qk(pstr, qt, 128, bq_t, 0)
        emit_qg(0, 2, pssca, pstr)
        emit_qg(1, 2, pssca, pstr)
    # Phase 2 (q-groups 1..7): all inputs resident; the freed transient
    # banks widen the score tiles to [128, 3, 512] so each exp instruction
    # covers 1536 elements (fewer instructions, less fixed overhead on the
    # bottleneck ScalarE).
    with tc.tile_pool(name="psscb", bufs=2, space="PSUM") as psscb:
        # Bank-reuse barrier: phase-2 score tiles recycle the banks of
        # phase-1's score/transient pools.  The allocator is lifetime-
        # scoped but execution pipelines across the phases, so gate
        # phase-2's first PE write behind q-group 0's final exp (the last
        # reader of those banks): token copy (waits the final exp) ->
        # barrier matmul (reads the token range; queues ahead of all
        # phase-2 matmuls in the in-order PE stream) -> tiny consumer so
        # the chain is not dead-code-eliminated.
        nc.vector.tensor_copy(dummy[:, 2:3], ring[:, 64 * QW - 1:64 * QW])
        bar = psscb.tile([128, 3, QW], f32, name="ps_sc", tag="sc")
        nc.tensor.matmul(bar[0:64, 0:1, 0:64], lhsT=dummy[:, 0:64],
                         rhs=dummy[:, 0:64], start=True, stop=True)
        nc.vector.tensor_copy(dummy[0:64, 3:4], bar[0:64, 0:1, 0:1])
        for qg in range(2, qg_n - 1):
            emit_qg(qg, FDU2, psscb, None)
        emit_qg(qg_n - 1, FDU2, psscb, None, stagger=True)


def _get_nc(s=S):
    if s not in _CACHE:
        _CACHE[s] = _build(s)
    return _CACHE[s]


def _shard_inputs(x, Wq, bq, Wk, bk, Wv, bv):
    import ml_dtypes

    bf16 = ml_dtypes.bfloat16
    f32 = np.float32
    ident = np.eye(128, dtype=f32)
    xTb = [np.ascontiguousarray(x[b].T).astype(bf16) for b in range(B)]
    xAb = [np.ascontiguousarray(
        np.concatenate([xTb[b][c * 128:(c + 1) * 128, 0:512]
                        for c in range(4)], axis=1)) for b in range(B)]
    wpacks, cpacks = [], []
    for hp in range(4):
        r = slice(128 * hp, 128 * hp + 128)
        blocks = []
        for Wm in (Wk[r], Wq[r] * 0.125, Wv[r]):
            WmT = Wm.T  # [512, 128]
            blocks += [WmT[c * 128:(c + 1) * 128] for c in range(4)]
        wpk = np.concatenate(blocks, axis=1)  # [128, 1536], K|Q|V major
        wpacks.append(np.ascontiguousarray(wpk).astype(bf16))
        cpk = np.concatenate(
            [bk[r].reshape(128, 1), (bq[r] * 0.125).reshape(128, 1),
             np.tile(bv[r][None, :], (128, 1)), ident], axis=1)  # [128, 258]
        cpacks.append(np.ascontiguousarray(cpk).astype(f32))
    in_maps = []
    for c in range(8):
        b, hp = divmod(c, 4)
        in_maps.append({
            "xT": xTb[b],
            "xA": xAb[b],
            "wpack": wpacks[hp],
            "cpack": cpacks[hp],
        })
    return in_maps


def kernel(x, Wq, bq, Wk, bk, Wv, bv, _trace=False):
    from concourse.bass_utils import run_bass_kernel_spmd

    x = np.asarray(x, dtype=np.float32)
    Wq = np.asarray(Wq, dtype=np.float32)
    bq = np.asarray(bq, dtype=np.float32)
    Wk = np.asarray(Wk, dtype=np.float32)
    bk = np.asarray(bk, dtype=np.float32)
    Wv = np.asarray(Wv, dtype=np.float32)
    bv = np.asarray(bv, dtype=np.float32)

    nc = _get_nc(S)
    in_maps = _shard_inputs(x, Wq, bq, Wk, bk, Wv, bv)
    try:
        res = run_bass_kernel_spmd(nc, in_maps, core_ids=list(range(8)),
                                   trace=_trace)
    except (ModuleNotFoundError, ImportError):
        # Tracing requested (arg or BASS_TRACE env) but this axon client has
        # no NTFF profiling hooks -- rerun untraced.
        import os
        os.environ["BASS_NEVER_TRACE"] = "1"
        res = run_bass_kernel_spmd(nc, in_maps, core_ids=list(range(8)),
                                   trace=False)
    kernel._last_results = res

    out = np.empty((B, S, D), dtype=np.float32)
    for c in range(8):
        b, hp = divmod(c, 4)
        out[b, :, 128 * hp:128 * hp + 128] = res.results[c]["out"]
    return out


# revision 12
# speedup vs baseline: 1.3106x; 1.0021x over previous
"""Multi-head attention (B=2, S=4096, D=512, H=8, HD=64, fp32) on 8 TRN2 cores.

Same math and sharding (core c -> batch c//4,
head pair c%4), restructured schedule:

  * consts/weights/x-block-0 DMA'd first in single dispatches (wpack
    [128,1536] bf16 packed K|Q|V-major so K's weights land first, xA
    [128,2048] bf16 = the four partition-chunks of x cols 0:512 side by
    side, cpack [128,258] f32) so the first projection is not queued behind
    the 4 MB xT transfer; V-tile memsets go to the idle GpSimd.
  * two-phase PSUM budget (Tile pools are lifetime-scoped, so a closed
    pool's banks are recycled): phase 1 = q-groups 0-1 with narrow
    [128,2,512]x2 score tiles (4 banks) + a 2-slot transient pool for the
    PE-clock warmup and the just-in-time K/V/Q projections (K and V stream
    inside qg0's k-chunk loop; Q blocks 2-7 inside qg1's, where the PE has
    slack); phase 2 = q-groups 2-7 with [128,3,512]x2 score tiles (6
    banks), so each exp instruction covers 1536 elements - fewer
    instructions on the bottleneck ScalarE.  A token-dependency barrier
    gates phase 2's first PE write behind phase 1's final exp (the bank
    recycling is safe only once the last reader retires).
  * exp output streams into a persistent full-q-group fp8 ring
    [128, 32768]; attention*V uses fp8(e4m3) DoubleRow matmuls reading
    kc-pair regions as [128, 2, 512] views (AP.rearrange), V stored fp8 as
    [128, 32, 80] with col 64 = 1.0 (the softmax row-sums ride along in the
    matmul), one matmul per kc-pair per head (2 MACs/cell/cycle).
  * the last q-group is head-staggered (head 1's 32 k-chunks sweep first),
    so head 1's transpose/normalize tail hides under head 0's exps and
    only half a tail remains exposed after the final exp.
"""

import numpy as np

B, S, D, H = 2, 4096, 512, 8
HD = D // H          # 64
OD = 128             # output dims per core (2 heads)
QW = 512             # query group width

_CACHE = {}


def _build(s=S, rep=1, dr=True, loop=1, empty=False, fdu2=3):
    import concourse.bacc as bacc
    import concourse.mybir as mybir
    import concourse.tile as tile

    f32 = mybir.dt.float32
    bf16 = mybir.dt.bfloat16
    f8 = mybir.dt.float8e4

    nc = bacc.Bacc(None, target_bir_lowering=False)

    xT = nc.dram_tensor("xT", [D, s], bf16, kind="ExternalInput")
    # xA: the four 128-partition chunks of x's first 512 columns packed
    # side by side, so block 0 lands in ONE DMA dispatch; wpack: all four
    # weight chunks packed likewise.
    xA = nc.dram_tensor("xA", [128, 4 * QW], bf16, kind="ExternalInput")
    wpack = nc.dram_tensor("wpack", [128, 4 * 384], bf16,
                           kind="ExternalInput")
    cpack = nc.dram_tensor("cpack", [128, 258], f32, kind="ExternalInput")
    out = nc.dram_tensor("out", [s, OD], f32, kind="ExternalOutput")

    with tile.TileContext(nc) as tc:
        with (
            tc.tile_pool(name="persist", bufs=1) as persist,
            tc.tile_pool(name="outsb", bufs=8) as outsb,
            tc.tile_pool(name="outt", bufs=4) as outtp,
            tc.tile_pool(name="psav", bufs=2, space="PSUM") as psav,
        ):
            Exp = mybir.ActivationFunctionType.Exp
            dummy = persist.tile([128, 64], bf16, name="dummy", tag="dummy")
            nc.vector.memset(dummy[:], 0.0)
            # Warm the exp table during input DMA.
            warm = persist.tile([1, 1], f32, name="warm", tag="warm")
            nc.vector.memset(warm[:], 0.0)
            nc.scalar.activation(warm[:], warm[:], Exp)
            # Warm the PE (HAM clock ramps to 2.4 GHz only after ~3us of
            # continuous busy): stream dummy matmuls on a zeroed tile while
            # the input DMAs are in flight, so the first projections run at
            # full clock.  Two chains occupy BOTH transient-pool slots, so
            # the first projection's PSUM allocation (and thus everything
            # after it) queues behind the warmup instead of the scheduler
            # scattering the dummies as gap fillers.  The tiny copies keep
            # the chains from being DCE'd.
            # ---- consts + x block 0 first (one dispatch each), then x ----
            wp_all = persist.tile([128, 4 * 384], bf16, name="wp_all",
                                  tag="wp")
            xa_sb = persist.tile([128, 4 * QW], bf16, name="xa", tag="xa")
            # split so K's weights + x block 0 (the first projection's
            # operands) land before Q/V weights
            nc.sync.dma_start(wp_all[:, 0:512], wpack[:, 0:512])
            nc.sync.dma_start(xa_sb[:], xA[:])
            nc.sync.dma_start(wp_all[:, 512:1024], wpack[:, 512:1024])
            nc.sync.dma_start(wp_all[:, 1024:1536], wpack[:, 1024:1536])
            cp = persist.tile([128, 258], f32, name="cp", tag="cp")
            nc.sync.dma_start(cp[:], cpack[:])
            bk_t = cp[:, 0:1]
            bq_t = cp[:, 1:2]
            bvb_t = cp[:, 2:130]
            id_t = cp[:, 130:258]

            xt = [persist.tile([128, s], bf16, name=f"xt{c}", tag=f"xt{c}")
                  for c in range(4)]
            h2 = (s - QW) // 2 + QW
            for lo, hi in ((QW, h2), (h2, s)):
                for c in range(4):
                    nc.sync.dma_start(xt[c][:, lo:hi],
                                      xT[c * 128:(c + 1) * 128, lo:hi])

            qt = persist.tile([128, s], bf16, name="qt", tag="qt")
            kt = persist.tile([128, s], bf16, name="kt", tag="kt")
            # full-q-group exp ring: every exp instruction writes a
            # contiguous span; attention*V reads kc-pair regions as
            # [128, 2, 512] fp8 views for DoubleRow
            ring = persist.tile([128, 64 * QW], f8, name="ring", tag="ring")
            if dr:
                v_sb = [persist.tile([128, 32, 80], f8, name=f"vsb{h}",
                                     tag=f"vsb{h}") for h in (0, 1)]
            else:
                v_sb = [persist.tile([128, 32, 65], bf16, name=f"vsb{h}",
                                     tag=f"vsb{h}") for h in (0, 1)]
            for h in (0, 1):
                nc.any.memset(v_sb[h][:], 1.0)

            def body():
                _emit_body(nc, tc, mybir, s, dr, qt, kt, xt, xa_sb, wp_all,
                           bq_t, bk_t, bvb_t, id_t, v_sb, ring, out,
                           outsb, outtp, psav, dummy, fdu2)

            if loop > 1:
                # Hardware loop around the body: used only for benchmarking
                # (re-runs the identical computation; amplifies kernel time
                # over the per-call host/transfer overhead).  empty=True
                # keeps only a token op in the loop, to measure the For_i
                # per-iteration barrier overhead for subtraction.
                with tc.For_i(0, loop, 1):
                    if empty:
                        nc.vector.memset(warm[:], 0.0)
                    else:
                        body()
            else:
                for _ in range(rep):
                    body()

    nc.compile()
    return nc


def _emit_body(nc, tc, mybir, s, dr, qt, kt, xt, xa_sb, wp_all, bq_t, bk_t,
               bvb_t, id_t, v_sb, ring, out, outsb, outtp, psav, dummy,
               FDU2=3):
    f32 = mybir.dt.float32
    bf16 = mybir.dt.bfloat16
    f8 = mybir.dt.float8e4
    Exp = mybir.ActivationFunctionType.Exp
    DR = mybir.MatmulPerfMode.DoubleRow
    qg_n = s // QW
    kc_n = s // 128

    def proj_qk(pool, dst, wcol, b_t, sb):
        ps = pool.tile([128, QW], f32, name="ps_tr", tag="tr")
        for c in range(4):
            rhs = (xa_sb[:, c * QW:(c + 1) * QW] if sb == 0 else
                   xt[c][:, sb * QW:(sb + 1) * QW])
            nc.tensor.matmul(
                ps[:],
                lhsT=wp_all[:, wcol * 4 + c * 128:wcol * 4 + (c + 1) * 128],
                rhs=rhs,
                start=(c == 0),
                stop=(c == 3),
            )
        nc.vector.tensor_scalar_add(
            dst[:, sb * QW:(sb + 1) * QW], ps[:], b_t
        )

    def vchunk(pool, j):
        ps = pool.tile([128, 128], f32, name="ps_tr", tag="tr")
        for c in range(4):
            lhsT = (xa_sb[:, c * QW + j * 128:c * QW + (j + 1) * 128]
                    if j < 4 else xt[c][:, j * 128:(j + 1) * 128])
            nc.tensor.matmul(
                ps[:],
                lhsT=lhsT,
                rhs=wp_all[:, 1024 + c * 128:1024 + (c + 1) * 128],
                start=(c == 0),
                stop=(c == 3),
            )
        for h in (0, 1):
            nc.vector.tensor_add(
                v_sb[h][:, j:j + 1, 0:64],
                ps[:, h * 64:(h + 1) * 64],
                bvb_t[:, h * 64:(h + 1) * 64],
            )

    def emit_qg(qg, fdu, pool, trpool, stagger=False):
        """One 512-query group: scores -> exp (fdu units per instruction,
        written into the contiguous exp ring) -> DoubleRow attention*V per
        kc pair.  qg 0 additionally JIT-emits the K/V projections and ALL
        remaining Q-block projections (the later groups' pool has no
        transient slots - their banks belong to the wider score tiles)."""
        q0 = qg * QW
        if stagger:
            # head-major unit order (all of head 1, then head 0): head 1's
            # attention*V completes mid-group, so its output tail hides
            # under head 0's remaining exps and only head 0's tail is
            # exposed after the final exp.  Allocate av[1] first so head
            # 1's transposes can recycle its slot without waiting on the
            # still-accumulating av[0].
            av1 = psav.tile([65, QW], f32, name="av", tag="av")
            av0 = psav.tile([65, QW], f32, name="av", tag="av")
            av = [av0, av1]
            order = [(kc, 1) for kc in range(kc_n)] + \
                    [(kc, 0) for kc in range(kc_n)]
        else:
            av = [psav.tile([65, QW], f32, name="av", tag="av")
                  for _ in (0, 1)]
            order = [(kc, h) for kc in range(kc_n) for h in (0, 1)]
        cur = None
        exp_off = 0
        exp_units = 0
        next_pair = 0
        tiles = {}
        n_units = 2 * kc_n
        last = (qg == qg_n - 1)
        # one [128, 4(blk), OD] tile so the whole group's output goes out
        # in a single DMA dispatch (four separate dispatches serialize at
        # ~625ns each on the HWDGE queue - ~1.9us exposed on the final
        # group's tail)
        ot = outsb.tile([128, 4, OD], f32, name="ot", tag="outsb")
        tail_done = {0: False, 1: False}

        def tail_head(h):
            # copy (releasing av[h]'s slot), then per-block transpose +
            # normalize.  In the staggered last group head 1's tail runs
            # mid-group on the DVE (ScalarE is still busy with head 0's
            # exps); the final head's copy uses the then-idle ScalarE.
            tail_done[h] = True
            outt = outtp.tile([65, QW], f32, name="outt", tag="outt")
            if last and not stagger or (stagger and h == 0):
                nc.scalar.copy(outt[:], av[h][:])
            elif last and stagger and h == 1:
                nc.vector.tensor_copy(outt[:], av[h][:])
            elif last:
                nc.scalar.copy(outt[:], av[h][:])
            else:
                nc.vector.tensor_copy(outt[:], av[h][:])
            for blk in range(4):
                tp = psav.tile([128, 65], f32, name="tp", tag="av")
                nc.tensor.transpose(
                    tp[:],
                    outt[:, blk * 128:(blk + 1) * 128],
                    id_t[0:65, 0:65],
                )
                rs = outsb.tile([128, 1], f32, name="rs", tag="rs")
                nc.vector.reciprocal(rs[:], tp[:, 64:65])
                nc.vector.tensor_scalar_mul(
                    ot[:, blk:blk + 1, h * HD:(h + 1) * HD],
                    tp[:, 0:64], rs[:]
                )

        def emit_ready(done):
            nonlocal exp_off, exp_units, next_pair
            while (exp_units + fdu <= done or
                   (done == n_units and exp_units < done)):
                n = min(fdu, done - exp_units)
                cur_e = tiles.pop(exp_units // fdu)
                nc.scalar.activation(
                    ring[:, exp_off:exp_off + n * QW],
                    cur_e[:, 0:n, :], Exp)
                exp_off += n * QW
                exp_units += n
                if stagger:
                    while (next_pair + 1) * 2 <= exp_units:
                        P = next_pair
                        hh = 1 if P < kc_n // 2 else 0
                        pih = P % (kc_n // 2)
                        reg = ring[:, P * 1024:(P + 1) * 1024].rearrange(
                            "p (a q) -> p a q", a=2)
                        nc.tensor.matmul(
                            av[hh][:],
                            lhsT=v_sb[hh][:, 2 * pih:2 * pih + 2, 0:65],
                            rhs=reg[:],
                            start=(pih == 0),
                            stop=(pih == kc_n // 2 - 1),
                            perf_mode=DR,
                        )
                        next_pair += 1
                else:
                    while (next_pair + 1) * 4 <= exp_units:
                        p = next_pair
                        reg = ring[:, p * 2048:(p + 1) * 2048].rearrange(
                            "p (a q) -> p a q", a=2)
                        for hh in (0, 1):
                            nc.tensor.matmul(
                                av[hh][:],
                                lhsT=v_sb[hh][:, 2 * p:2 * p + 2, 0:65],
                                rhs=reg[:, :, hh * QW:(hh + 1) * QW],
                                start=(p == 0),
                                stop=(p == kc_n // 2 - 1),
                                perf_mode=DR,
                            )
                        next_pair += 1

        for u, (kc, h) in enumerate(order):
            if u % fdu == 0:
                cur = pool.tile([128, fdu, QW], f32, name="ps_sc",
                                tag="sc")
                tiles[u // fdu] = cur
            nc.tensor.matmul(
                cur[:, (u % fdu):(u % fdu) + 1, :],
                lhsT=kt[h * HD:(h + 1) * HD, kc * 128:(kc + 1) * 128],
                rhs=qt[h * HD:(h + 1) * HD, q0:q0 + QW],
                start=True,
                stop=True,
            )
            # JIT projections sit between the scores and the exp/AV stream:
            # they must precede the AV that consumes them (emission order IS
            # dependency order) without delaying the next kc's scores.
            if trpool is not None and h == 1:
                if qg == 0:
                    if kc % 4 == 2 and kc < 28:
                        proj_qk(trpool, kt, 0, bk_t, kc // 4 + 1)
                    if kc < 2:
                        vchunk(trpool, kc)
                    if kc < kc_n - 2:
                        vchunk(trpool, kc + 2)
                    if kc == 4:
                        proj_qk(trpool, qt, 128, bq_t, 1)
                else:
                    # q-group 1 carries the remaining Q-block projections
                    # (q-group 0's PE is already saturated with K/V JIT)
                    if kc >= 1 and (kc - 1) % 3 == 0 and (kc - 1) // 3 < 6:
                        proj_qk(trpool, qt, 128, bq_t, (kc - 1) // 3 + 2)
            # exps covering completed units, then AV for completed pairs
            emit_ready(u + 1)
            if stagger and next_pair >= kc_n // 2 and not tail_done[1]:
                tail_head(1)
        # end-of-group tail: remaining heads, then one fused store
        for h in (0, 1):
            if not tail_done[h]:
                tail_head(h)
        dst = out[q0:q0 + QW, :].rearrange("(b p) c -> p b c", p=128)
        nc.sync.dma_start(dst, ot[:])

    # Phase 1 (q-group 0): narrow score tiles + transient pool for the PE
    # warmup and the JIT K/V/Q projections.
    with (
        tc.tile_pool(name="pssca", bufs=2, space="PSUM") as pssca,
        tc.tile_pool(name="pstr", bufs=2, space="PSUM") as pstr,
    ):
        for ch in range(2):
            dps = pstr.tile([128, 64], f32, name="ps_tr", tag="tr")
            for i in range(28):
                nc.tensor.matmul(dps[0:64, :], lhsT=dummy[:], rhs=dummy[:],
                                 start=(i == 0), stop=(i == 27))
            nc.vector.tensor_copy(dummy[:, ch:ch + 1], dps[:, 0:1])
        proj_qk(pstr, kt, 0, bk_t, 0)
        proj_qk(pstr, qt, 128, bq_t, 0)
        emit_qg(0, 2, pssca, pstr)
        emit_qg(1, 2, pssca, pstr)
    # Phase 2 (q-groups 1..7): all inputs resident; the freed transient
    # banks widen the score tiles to [128, 3, 512] so each exp instruction
    # covers 1536 elements (fewer instructions, less fixed overhead on the
    # bottleneck ScalarE).
    with tc.tile_pool(name="psscb", bufs=2, space="PSUM") as psscb:
        # Bank-reuse barrier: phase-2 score tiles recycle the banks of
        # phase-1's score/transient pools.  The allocator is lifetime-
        # scoped but execution pipelines across the phases, so gate
        # phase-2's first PE write behind q-group 0's final exp (the last
        # reader of those banks): token copy (waits the final exp) ->
        # barrier matmul (reads the token range; queues ahead of all
        # phase-2 matmuls in the in-order PE stream) -> tiny consumer so
        # the chain is not dead-code-eliminated.
        nc.vector.tensor_copy(dummy[:, 2:3], ring[:, 64 * QW - 1:64 * QW])
        bar = psscb.tile([128, 3, QW], f32, name="ps_sc", tag="sc")
        nc.tensor.matmul(bar[0:64, 0:1, 0:64], lhsT=dummy[:, 0:64],
                         rhs=dummy[:, 0:64], start=True, stop=True)
        nc.vector.tensor_copy(dummy[0:64, 3:4], bar[0:64, 0:1, 0:1])
        for qg in range(2, qg_n - 1):
            emit_qg(qg, FDU2, psscb, None)
        emit_qg(qg_n - 1, FDU2, psscb, None, stagger=True)


def _get_nc(s=S):
    if s not in _CACHE:
        _CACHE[s] = _build(s)
    return _CACHE[s]


def _shard_inputs(x, Wq, bq, Wk, bk, Wv, bv):
    import ml_dtypes

    bf16 = ml_dtypes.bfloat16
    f32 = np.float32
    ident = np.eye(128, dtype=f32)
    xTb = [np.ascontiguousarray(x[b].T).astype(bf16) for b in range(B)]
    xAb = [np.ascontiguousarray(
        np.concatenate([xTb[b][c * 128:(c + 1) * 128, 0:512]
                        for c in range(4)], axis=1)) for b in range(B)]
    wpacks, cpacks = [], []
    for hp in range(4):
        r = slice(128 * hp, 128 * hp + 128)
        blocks = []
        for Wm in (Wk[r], Wq[r] * 0.125, Wv[r]):
            WmT = Wm.T  # [512, 128]
            blocks += [WmT[c * 128:(c + 1) * 128] for c in range(4)]
        wpk = np.concatenate(blocks, axis=1)  # [128, 1536], K|Q|V major
        wpacks.append(np.ascontiguousarray(wpk).astype(bf16))
        cpk = np.concatenate(
            [bk[r].reshape(128, 1), (bq[r] * 0.125).reshape(128, 1),
             np.tile(bv[r][None, :], (128, 1)), ident], axis=1)  # [128, 258]
        cpacks.append(np.ascontiguousarray(cpk).astype(f32))
    in_maps = []
    for c in range(8):
        b, hp = divmod(c, 4)
        in_maps.append({
            "xT": xTb[b],
            "xA": xAb[b],
            "wpack": wpacks[hp],
            "cpack": cpacks[hp],
        })
    return in_maps


def kernel(x, Wq, bq, Wk, bk, Wv, bv, _trace=False):
    from concourse.bass_utils import run_bass_kernel_spmd

    x = np.asarray(x, dtype=np.float32)
    Wq = np.asarray(Wq, dtype=np.float32)
    bq = np.asarray(bq, dtype=np.float32)
    Wk = np.asarray(Wk, dtype=np.float32)
    bk = np.asarray(bk, dtype=np.float32)
    Wv = np.asarray(Wv, dtype=np.float32)
    bv = np.asarray(bv, dtype=np.float32)

    nc = _get_nc(S)
    in_maps = _shard_inputs(x, Wq, bq, Wk, bk, Wv, bv)
    try:
        res = run_bass_kernel_spmd(nc, in_maps, core_ids=list(range(8)),
                                   trace=_trace)
    except (ModuleNotFoundError, ImportError):
        # Tracing requested (arg or BASS_TRACE env) but this axon client has
        # no NTFF profiling hooks -- rerun untraced.
        import os
        os.environ["BASS_NEVER_TRACE"] = "1"
        res = run_bass_kernel_spmd(nc, in_maps, core_ids=list(range(8)),
                                   trace=False)
    kernel._last_results = res

    out = np.empty((B, S, D), dtype=np.float32)
    for c in range(8):
        b, hp = divmod(c, 4)
        out[b, :, 128 * hp:128 * hp + 128] = res.results[c]["out"]
    return out
